# revision 5
# baseline (speedup 1.0000x reference)
"""LIF neuron (leaky integrate-and-fire) Bass kernel for Trainium2.

Reference semantics (per element, recurrence over time axis T=32):
    mem_t   = tau * mem_{t-1} + x_t
    spike_t = 1.0 if mem_t > vth else 0.0
    mem_t   = mem_t * (1 - spike_t)        # hard reset

Input  x: [16, 32, 65536] f32  ->  Output spikes: [16, 32, 65536] f32.

Sharding: pure data parallel over batch. 8 cores x 2 batch rows each.
Per core each timestep is a [128, 1024] f32 tile (2 batches x 512
d-elements per partition). Per step:
  DVE  scalar_tensor_tensor: acc = (mem * tau) + x_t          (1x fp32)
  ACT  Sign:  sgn = sign(acc - vth)                            (in {-1,0,1})
  ACT  Relu:  spk = relu(sgn)                                  (in {0,1})
  DVE  scalar_tensor_tensor: mem' = (acc <= vth) * acc         (hard reset)
DMA: 4-step groups, 2 MiB loads (sync engine ring), 1 MiB stores
(scalar engine ring) so loads and stores don't share one HWDGE FIFO.

Engine budgets per core: DMA ~94us (roofline @ ~358 GB/s), DVE ~77us,
ACT ~73us -> DMA bound.
"""

import sys

sys.path.insert(0, "/opt/trn_rl_repo")

import numpy as np

from concourse import bacc, mybir, tile
from concourse.bass_utils import run_bass_kernel_spmd

TAU = 0.2
VTH = 0.5

B, T, D = 16, 32, 65536
N_CORES = 8
B_SH = B // N_CORES          # 2 batch rows per core
P = 128                      # SBUF partitions
F = B_SH * D // P            # 1024 free elems per step-tile
FB = D // P                  # 512 free elems per batch row
GS = 4                       # timesteps per DMA group
NG = T // GS                 # 8 groups

_prog = None


def _build_program():
    f32 = mybir.dt.float32
    nc = bacc.Bacc(
        "TRN2",
        target_bir_lowering=False,
        debug=False,
        enable_asserts=False,
        num_devices=N_CORES,
    )
    x = nc.dram_tensor("x", [B_SH, T, D], f32, kind="ExternalInput").ap()
    out = nc.dram_tensor("out", [B_SH, T, D], f32, kind="ExternalOutput").ap()

    # [g, p, tl, b, f]: group, partition, step-in-group, batch, free
    xr = x.rearrange("b (g tl) (p f) -> g p tl b f", tl=GS, p=P)
    outr = out.rearrange("b (g tl) (p f) -> g p tl b f", tl=GS, p=P)

    mult = mybir.AluOpType.mult
    add = mybir.AluOpType.add
    is_le = mybir.AluOpType.is_le
    Sign = mybir.ActivationFunctionType.Sign
    Relu = mybir.ActivationFunctionType.Relu

    with tile.TileContext(nc) as tc:
        with (
            tc.tile_pool(name="xt", bufs=3) as xp,
            tc.tile_pool(name="spk", bufs=3) as sp,
            tc.tile_pool(name="acc", bufs=3) as ap_,
            tc.tile_pool(name="sgn", bufs=3) as gp,
            tc.tile_pool(name="mem", bufs=2) as mp,
            tc.tile_pool(name="const", bufs=1) as cp,
        ):
            nvth = cp.tile([P, 1], f32)
            nc.gpsimd.memset(nvth[:], -VTH)
            mem = None
            for g in range(NG):
                xt = xp.tile([P, GS * F], f32)
                xt_v = xt[:].rearrange("p (tl b f) -> p tl b f", tl=GS, b=B_SH)
                for b in range(B_SH):
                    # 1 MiB load per batch row: [p:128][tl:4][f:512]
                    nc.sync.dma_start(out=xt_v[:, :, b], in_=xr[g][:, :, b])
                spk = sp.tile([P, GS * F], f32)
                for tl in range(GS):
                    t = g * GS + tl
                    xs = xt[:, tl * F : (tl + 1) * F]
                    if t == 0:
                        acc = xs  # mem_{-1} = 0 -> acc = x_0
                    else:
                        acc = ap_.tile([P, F], f32)
                        # acc = (mem * tau) + x_t
                        nc.vector.scalar_tensor_tensor(
                            out=acc[:], in0=mem[:], scalar=TAU, in1=xs,
                            op0=mult, op1=add,
                        )
                    sgn = gp.tile([P, F], f32)
                    # sgn = sign(acc - vth); relu(sgn) = (acc > vth) exactly
                    nc.scalar.activation(sgn[:], acc[:], Sign, bias=nvth[:])
                    nc.scalar.activation(spk[:, tl * F : (tl + 1) * F], sgn[:], Relu)
                    mem = mp.tile([P, F], f32)
                    # mem' = (acc <= vth) * acc   (hard reset)
                    nc.vector.scalar_tensor_tensor(
                        out=mem[:], in0=acc[:], scalar=VTH, in1=acc[:],
                        op0=is_le, op1=mult,
                    )
                # 1 MiB store per batch row; issue from scalar engine so
                # stores ride qActDynamicHW while loads ride qSPDynamicHW
                spk_v = spk[:].rearrange("p (tl b f) -> p tl b f", tl=GS, b=B_SH)
                for b in range(B_SH):
                    nc.scalar.dma_start(out=outr[g][:, :, b], in_=spk_v[:, :, b])
    nc.compile()
    return nc


def _get_program():
    global _prog
    if _prog is None:
        _prog = _build_program()
    return _prog


def _shard(x):
    return [
        {"x": np.ascontiguousarray(x[i * B_SH : (i + 1) * B_SH])}
        for i in range(N_CORES)
    ]


def kernel(x):
    x = np.asarray(x, dtype=np.float32)
    assert x.shape == (B, T, D), x.shape
    nc = _get_program()
    res = run_bass_kernel_spmd(nc, _shard(x), list(range(N_CORES)))
    return np.concatenate(
        [res.results[i]["out"] for i in range(N_CORES)], axis=0
    )


# revision 9
# speedup vs baseline: 778.5915x; 778.5915x over previous
"""LIF neuron (leaky integrate-and-fire) Bass kernel for Trainium2.

Reference semantics (per element, recurrence over time axis T=32):
    mem_t   = tau * mem_{t-1} + x_t
    spike_t = 1.0 if mem_t > vth else 0.0
    mem_t   = mem_t * (1 - spike_t)        # hard reset

Input  x: [16, 32, 65536] f32  ->  Output spikes: [16, 32, 65536] f32.

Sharding: pure data parallel over batch. 8 cores x 2 batch rows each.
Per core each timestep is a [128, 1024] f32 tile (2 batches x 512
d-elements per partition). Per step:
  DVE  scalar_tensor_tensor: acc = (mem * tau) + x_t          (1x fp32)
  ACT  Sign:  sgn = sign(acc - vth)                            (in {-1,0,1})
  ACT  Relu:  spk = relu(sgn)                                  (in {0,1})
  DVE  scalar_tensor_tensor: mem' = (acc <= vth) * acc         (hard reset)
DMA: 4-step groups, 2 MiB loads (sync engine ring), 1 MiB stores
(scalar engine ring) so loads and stores don't share one HWDGE FIFO.

Engine budgets per core: DMA ~94us (roofline @ ~358 GB/s), DVE ~77us,
ACT ~73us -> DMA bound.
"""

import sys

sys.path.insert(0, "/opt/trn_rl_repo")

import numpy as np

from concourse import bacc, mybir, tile
from concourse.bass_utils import run_bass_kernel_spmd

TAU = 0.2
VTH = 0.5

B, T, D = 16, 32, 65536
N_CORES = 8
B_SH = B // N_CORES          # 2 batch rows per core
P = 128                      # SBUF partitions
F = B_SH * D // P            # 1024 free elems per step-tile
FB = D // P                  # 512 free elems per batch row
GS = 4                       # timesteps per DMA group
NG = T // GS                 # 8 groups

_progs = {}


def _build_program(hw_loop=None):
    f32 = mybir.dt.float32
    nc = bacc.Bacc(
        "TRN2",
        target_bir_lowering=False,
        debug=False,
        enable_asserts=False,
        num_devices=N_CORES,
    )
    x = nc.dram_tensor("x", [B_SH, T, D], f32, kind="ExternalInput").ap()
    out = nc.dram_tensor("out", [B_SH, T, D], f32, kind="ExternalOutput").ap()

    # [g, p, tl, b, f]: group, partition, step-in-group, batch, free
    xr = x.rearrange("b (g tl) (p f) -> g p tl b f", tl=GS, p=P)
    outr = out.rearrange("b (g tl) (p f) -> g p tl b f", tl=GS, p=P)

    with tile.TileContext(nc) as tc:
        with (
            tc.tile_pool(name="xt", bufs=3) as xp,
            tc.tile_pool(name="spk", bufs=3) as sp,
            tc.tile_pool(name="acc", bufs=3) as ap_,
            tc.tile_pool(name="sgn", bufs=3) as gp,
            tc.tile_pool(name="mem", bufs=2) as mp,
            tc.tile_pool(name="const", bufs=1) as cp,
        ):
            nvth = cp.tile([P, 1], f32)
            nc.gpsimd.memset(nvth[:], -VTH)

            def body():
                one_pass(nc, tc, xr, outr, xp, sp, ap_, gp, mp, nvth)

            if hw_loop is None:
                body()
            else:
                # benchmarking only: repeat the full pass in a HW loop so
                # per-pass device time can be fit from wall-clock deltas
                with tc.For_i(0, hw_loop, 1):
                    body()
    nc.compile()
    return nc


def one_pass(nc, tc, xr, outr, xp, sp, ap_, gp, mp, nvth):
    f32 = mybir.dt.float32
    mult = mybir.AluOpType.mult
    add = mybir.AluOpType.add
    is_le = mybir.AluOpType.is_le
    Sign = mybir.ActivationFunctionType.Sign
    Relu = mybir.ActivationFunctionType.Relu
    mem = None
    if True:
        for g in range(NG):
                xt = xp.tile([P, GS * F], f32)
                xt_v = xt[:].rearrange("p (tl b f) -> p tl b f", tl=GS, b=B_SH)
                for b in range(B_SH):
                    # 1 MiB load per batch row: [p:128][tl:4][f:512]
                    nc.sync.dma_start(out=xt_v[:, :, b], in_=xr[g][:, :, b])
                spk = sp.tile([P, GS * F], f32)
                for tl in range(GS):
                    t = g * GS + tl
                    xs = xt[:, tl * F : (tl + 1) * F]
                    if t == 0:
                        acc = xs  # mem_{-1} = 0 -> acc = x_0
                    else:
                        acc = ap_.tile([P, F], f32)
                        # acc = (mem * tau) + x_t
                        nc.vector.scalar_tensor_tensor(
                            out=acc[:], in0=mem[:], scalar=TAU, in1=xs,
                            op0=mult, op1=add,
                        )
                    sgn = gp.tile([P, F], f32)
                    # sgn = sign(acc - vth); relu(sgn) = (acc > vth) exactly
                    nc.scalar.activation(sgn[:], acc[:], Sign, bias=nvth[:])
                    nc.scalar.activation(spk[:, tl * F : (tl + 1) * F], sgn[:], Relu)
                    mem = mp.tile([P, F], f32)
                    # mem' = (acc <= vth) * acc   (hard reset)
                    nc.vector.scalar_tensor_tensor(
                        out=mem[:], in0=acc[:], scalar=VTH, in1=acc[:],
                        op0=is_le, op1=mult,
                    )
                # 1 MiB store per batch row; issue from scalar engine so
                # stores ride qActDynamicHW while loads ride qSPDynamicHW
                spk_v = spk[:].rearrange("p (tl b f) -> p tl b f", tl=GS, b=B_SH)
                for b in range(B_SH):
                    nc.scalar.dma_start(out=outr[g][:, :, b], in_=spk_v[:, :, b])


def _get_program(hw_loop=None):
    if hw_loop not in _progs:
        _progs[hw_loop] = _build_program(hw_loop)
    return _progs[hw_loop]


def _shard(x):
    return [
        {"x": np.ascontiguousarray(x[i * B_SH : (i + 1) * B_SH])}
        for i in range(N_CORES)
    ]


def kernel(x):
    x = np.asarray(x, dtype=np.float32)
    assert x.shape == (B, T, D), x.shape
    nc = _get_program()
    res = run_bass_kernel_spmd(nc, _shard(x), list(range(N_CORES)))
    return np.concatenate(
        [res.results[i]["out"] for i in range(N_CORES)], axis=0
    )


# revision 29
# speedup vs baseline: 780.3073x; 1.0022x over previous
"""LIF neuron (leaky integrate-and-fire) Bass kernel for Trainium2.

Reference semantics (per element, recurrence over time axis T=32):
    mem_t   = tau * mem_{t-1} + x_t
    spike_t = 1.0 if mem_t > vth else 0.0
    mem_t   = mem_t * (1 - spike_t)        # hard reset

Input  x: [16, 32, 65536] f32  ->  Output spikes: [16, 32, 65536] f32.

Sharding: pure data parallel over batch. 8 cores x 2 batch rows each.
Per core each timestep is a [128, 1024] f32 tile (2 batches x 512
d-elements per partition). Per step:
  DVE  scalar_tensor_tensor: acc = (mem * tau) + x_t          (1x fp32)
  ACT  Sign:  sgn = sign(acc - vth)                            (in {-1,0,1})
  ACT  Relu:  spk = relu(sgn)                                  (in {0,1})
  DVE  scalar_tensor_tensor: mem' = (acc <= vth) * acc         (hard reset)
DMA: 4-step groups, 2 MiB loads (sync engine ring), 1 MiB stores
(scalar engine ring) so loads and stores don't share one HWDGE FIFO.

Engine budgets per core: DMA ~94us (roofline @ ~358 GB/s), DVE ~77us,
ACT ~73us -> DMA bound.
"""

import sys

sys.path.insert(0, "/opt/trn_rl_repo")

import numpy as np

from concourse import bacc, mybir, tile
from concourse.bass_utils import run_bass_kernel_spmd

TAU = 0.2
VTH = 0.5

B, T, D = 16, 32, 65536
N_CORES = 8
B_SH = B // N_CORES          # 2 batch rows per core
P = 128                      # SBUF partitions
F = B_SH * D // P            # 1024 free elems per step-tile
FB = D // P                  # 512 free elems per batch row
import os

GS = int(os.environ.get("LIF_GS", "4"))   # timesteps per DMA group
NG = T // GS                 # groups per pass

_progs = {}


def _build_program(hw_loop=None, mode="full"):
    f32 = mybir.dt.float32
    nc = bacc.Bacc(
        "TRN2",
        target_bir_lowering=False,
        debug=False,
        enable_asserts=False,
        num_devices=N_CORES,
    )
    x = nc.dram_tensor("x", [B_SH, T, D], f32, kind="ExternalInput").ap()
    out = nc.dram_tensor("out", [B_SH, T, D], f32, kind="ExternalOutput").ap()

    # [g, p, tl, b, f]: group, partition, step-in-group, batch, free
    xr = x.rearrange("b (g tl) (p f) -> g p tl b f", tl=GS, p=P)
    outr = out.rearrange("b (g tl) (p f) -> g p tl b f", tl=GS, p=P)

    with tile.TileContext(nc) as tc:
        xt_bufs = {4: 5, 8: 3, 2: 10}[GS]
        spk_bufs = {4: 5, 8: 2, 2: 10}[GS]
        with (
            tc.tile_pool(name="xt", bufs=xt_bufs) as xp,
            tc.tile_pool(name="spk", bufs=spk_bufs) as sp,
            tc.tile_pool(name="acc", bufs=3) as ap_,
            tc.tile_pool(name="sgn", bufs=3) as gp,
            tc.tile_pool(name="mem", bufs=2) as mp,
            tc.tile_pool(name="const", bufs=1) as cp,
        ):
            nvth = cp.tile([P, 1], f32)
            nc.gpsimd.memset(nvth[:], -VTH)

            def body():
                one_pass(nc, tc, xr, outr, xp, sp, ap_, gp, mp, nvth, mode)

            if hw_loop is None:
                body()
            else:
                # benchmarking only: repeat the full pass in a HW loop so
                # per-pass device time can be fit from wall-clock deltas
                with tc.For_i(0, hw_loop, 1):
                    body()
    nc.compile()
    return nc


def one_pass(nc, tc, xr, outr, xp, sp, ap_, gp, mp, nvth, mode="full"):
    do_dma = mode in ("full", "dma")
    do_compute = mode in ("full", "compute")
    f32 = mybir.dt.float32
    mult = mybir.AluOpType.mult
    add = mybir.AluOpType.add
    is_le = mybir.AluOpType.is_le
    Sign = mybir.ActivationFunctionType.Sign
    Relu = mybir.ActivationFunctionType.Relu
    mem = None
    if True:
        for g in range(NG):
                xt = xp.tile([P, GS * F], f32)
                xt_v = xt[:].rearrange("p (tl b f) -> p tl b f", tl=GS, b=B_SH)
                if do_dma and g == 0:
                    # head: per-step 256 KiB loads so step-0 compute starts
                    # after ~0.7us instead of waiting on a full 1 MiB load
                    for tl in range(GS):
                        for b in range(B_SH):
                            nc.sync.dma_start(
                                out=xt_v[:, tl, b], in_=xr[g][:, tl, b]
                            )
                elif do_dma:
                    for b in range(B_SH):
                        # 1 MiB load per batch row: [p:128][tl:4][f:512]
                        nc.sync.dma_start(out=xt_v[:, :, b], in_=xr[g][:, :, b])
                else:
                    # compute-only microbench: fabricate xt on the idle Pool
                    nc.gpsimd.memset(xt[:], 0.125)
                # dma-only microbench: no compute writes spk, so stores read
                # the loaded xt tile to keep a load->store dependency chain
                if do_compute:
                    spk = sp.tile([P, GS * F], f32, tag="spk")
                else:
                    spk = xt
                for tl in range(GS):
                    t = g * GS + tl
                    xs = xt[:, tl * F : (tl + 1) * F]
                    if t == 0:
                        acc = xs  # mem_{-1} = 0 -> acc = x_0
                    elif do_compute:
                        acc = ap_.tile([P, F], f32)
                        # acc = (mem * tau) + x_t
                        nc.vector.scalar_tensor_tensor(
                            out=acc[:], in0=mem[:], scalar=TAU, in1=xs,
                            op0=mult, op1=add,
                        )
                    if do_compute:
                        sgn = gp.tile([P, F], f32)
                        # sgn = sign(acc-vth); relu(sgn) = (acc > vth) exactly
                        nc.scalar.activation(sgn[:], acc[:], Sign, bias=nvth[:])
                        nc.scalar.activation(
                            spk[:, tl * F : (tl + 1) * F], sgn[:], Relu
                        )
                        mem = mp.tile([P, F], f32)
                        # mem' = (acc <= vth) * acc   (hard reset)
                        nc.vector.scalar_tensor_tensor(
                            out=mem[:], in0=acc[:], scalar=VTH, in1=acc[:],
                            op0=is_le, op1=mult,
                        )
                    if do_dma and tl % 2 == 1:
                        # store per 2-step half right after its relus land;
                        # issued from scalar engine so stores ride
                        # qActDynamicHW while loads ride qSPDynamicHW
                        spk_v = spk[:].rearrange(
                            "p (tl b f) -> p tl b f", tl=GS, b=B_SH
                        )
                        for b in range(B_SH):
                            nc.scalar.dma_start(
                                out=outr[g][:, tl - 1 : tl + 1, b],
                                in_=spk_v[:, tl - 1 : tl + 1, b],
                            )


def _get_program(hw_loop=None, mode="full"):
    key = (hw_loop, mode)
    if key not in _progs:
        _progs[key] = _build_program(hw_loop, mode)
    return _progs[key]


def _shard(x):
    return [
        {"x": np.ascontiguousarray(x[i * B_SH : (i + 1) * B_SH])}
        for i in range(N_CORES)
    ]


def kernel(x):
    x = np.asarray(x, dtype=np.float32)
    assert x.shape == (B, T, D), x.shape
    nc = _get_program()
    res = run_bass_kernel_spmd(nc, _shard(x), list(range(N_CORES)))
    return np.concatenate(
        [res.results[i]["out"] for i in range(N_CORES)], axis=0
    )
